# revision 30
# baseline (speedup 1.0000x reference)
"""Self-contained Trainium2 Bass kernel for 4-layer GraphSAGE (nn_LASAGE).

Strategy (v7):
  - Nodes dst-sharded across 8 cores (6250/core, padded to 6272 = 49 blocks of
    128). Aggregation POST-matmul: agg(x)@Wl == agg(x@Wl). Per layer each core
    computes y = h @ Wl for its shard; the Y table is replicated via two
    AllGathers (row groups a/b, split at 3136) in FP8-E4M3 (fp32 for the final
    64-wide table, gather elems must be 256B-aligned).
  - TWO-PASS layers: pass A accumulates group-a edge sums into an SBUF
    accumulator while group-b's AllGather (launched by the previous layer)
    completes underneath; pass B adds group-b sums, merges
    relu((sumA+sumB)*invdeg + x@Wr + b), and emits y_next + its AllGathers.
    Every AllGather thereby gets a full pass of compute cover.
  - Edge gathers: dma_gather, 7 tiles (896 idxs) per call (single-packet
    descriptor ceiling is 64/engine), calls cut across dst-block boundaries
    within a group, SWDGE queues matched post-scheduling to the Tile
    framework's DMASW sem lanes, 32KB descriptor carveout for ring depth.
  - Scatter-add via one-hot matmuls on the PE with EXACT {0,1} one-hots
    (is_equal only, fp8 out) -- the mean normalization happens once per block
    in the merge epilogue (invdeg replicated across partitions), never
    per-edge. fp8 feeds the PE directly (fp8 matmul == bf16 speed, exact
    psum accumulation of 0/1-weighted sums).
  - Everything else bf16 (x, h, weights) for FWL weight loads and 2x DVE.
    Biases ride the relu epilogue's per-partition bias AP.
"""
import sys, os, types

sys.path.insert(0, "/opt/trn_rl_repo")
import numpy as np

N = 50000
E = 800000
NCORES = 8
S = N // NCORES            # 6250 real nodes per core
SP = 6272                  # padded (49 blocks of 128)
NBLK = SP // 128
SPA = 3136                 # local rows in group a; 8*3136=25088 <= int16 max
SPB = SP - SPA
HROWS = [NCORES * SPA, NCORES * SPB]
AGBLK = (SPA + 127) // 128 - 1   # fire group-a AG after this block
D1 = 256
DM = 256
DO = 64
TPC = 7                    # tiles per call (896 idxs = 57 descs/engine <= 64 packet max)
CPC = TPC * 128 // 16


def _install_hooks():
    """antenv.axon_hooks shim so trace=True works in this image (optional)."""
    try:
        import antenv
        if "antenv.axon_hooks" not in sys.modules:
            mod = types.ModuleType("antenv.axon_hooks")
            mod._hook = None
            mod.set_axon_ntff_profile_hook = lambda h: setattr(mod, "_hook", h)
            mod.get_axon_ntff_profile_hook = lambda: mod._hook
            sys.modules["antenv.axon_hooks"] = mod
            antenv.axon_hooks = mod
        from antenv.axon_hooks import get_axon_ntff_profile_hook, set_axon_ntff_profile_hook
        if get_axon_ntff_profile_hook() is None:
            from trn_agent_boot.trn_boot import _ntff_profile_via_ctypes
            set_axon_ntff_profile_hook(_ntff_profile_via_ctypes("/opt/axon/libaxon_pjrt.so"))
        import concourse.bass_utils as bu
        bu.upload_artifacts = lambda tmpdir: f"file://{tmpdir}"
    except Exception:
        pass


def _preprocess(edge_index):
    """Edge lists per core, grouped by (src group, dst block), padded per-tile."""
    src = np.asarray(edge_index[0], np.int64)
    dst = np.asarray(edge_index[1], np.int64)
    core = dst // S
    dl = (dst % S).astype(np.int64)
    blk = dl // 128
    col = dl % 128
    sloc = src % S
    half = (sloc >= SPA).astype(np.int64)
    gsz = np.where(half == 0, SPA, SPB)
    grow = (src // S) * gsz + (sloc - half * SPA)   # row within its group table

    deg = np.bincount(core * S + dl, minlength=N).reshape(NCORES, S)

    order = np.lexsort((grow, blk, half, core))
    core_s, half_s, blk_s, col_s, row_s = (core[order], half[order], blk[order],
                                           col[order], grow[order])

    key = (core_s * 2 + half_s) * NBLK + blk_s
    counts = np.bincount(key, minlength=NCORES * 2 * NBLK).reshape(NCORES, 2, NBLK)
    tiles_hb = np.ceil(counts.max(axis=0) / 128).astype(np.int64)   # [2, NBLK]
    tiles_hb = np.maximum(tiles_hb, 1)

    pad_hb = tiles_hb * 128
    tot_h = pad_hb.sum(axis=1)
    seg_off = np.zeros((2, NBLK), np.int64)
    seg_off[:, 1:] = np.cumsum(pad_hb, axis=1)[:, :-1]

    srcpad = np.zeros((NCORES, 2), dtype=object)
    colpad = np.zeros((NCORES, 2), dtype=object)
    for c in range(NCORES):
        for h in range(2):
            srcpad[c, h] = np.zeros(int(tot_h[h]), np.int64)
            colpad[c, h] = np.full(int(tot_h[h]), -1, np.int64)
    grp = key
    first = np.r_[True, grp[1:] != grp[:-1]]
    gidx = np.arange(len(grp)) - np.maximum.accumulate(np.where(first, np.arange(len(grp)), 0))
    pos = seg_off[half_s, blk_s] + gidx
    for c in range(NCORES):
        m = core_s == c
        for h in range(2):
            mh = m & (half_s == h)
            p = pos[mh]
            srcpad[c, h][p] = row_s[mh]
            colpad[c, h][p] = col_s[mh]

    return {
        "tiles_hb": tiles_hb, "seg_off": seg_off,
        "srcpad": srcpad, "colpad": colpad, "deg": deg,
    }


def _build_callplan(tiles_hb):
    """Gather calls cut across block boundaries within a group."""
    ht = [int(tiles_hb[0].sum()), int(tiles_hb[1].sum())]
    calls = []
    call_base = [0, 0]
    for h in range(2):
        call_base[h] = len(calls)
        done = 0
        while done < ht[h]:
            k = min(TPC, ht[h] - done)
            calls.append(dict(h=h, k=k, t0=done))
            done += k
    # per-group per-block tile lists
    bt = [[[] for _ in range(NBLK)], [[] for _ in range(NBLK)]]
    for h in range(2):
        tstart = 0
        for b in range(NBLK):
            for j in range(int(tiles_hb[h, b])):
                t = tstart + j
                ci = call_base[h] + t // TPC
                slot = t % TPC
                dcol = (0 if h == 0 else ht[0]) + t
                bt[h][b].append((ci, slot, dcol))
            tstart += int(tiles_hb[h, b])
    return calls, bt, ht


def _idx_arrays(pre, calls, ht, core):
    """int16 idx image [128, ncalls*CPC] and dstloc [128, ntiles] f32."""
    ncalls = len(calls)
    tiles_total = ht[0] + ht[1]
    idx_img = np.zeros((16, ncalls * CPC), np.int16)
    dstloc = np.full((128, tiles_total), -1.0, np.float32)
    for ci, cl in enumerate(calls):
        h, k, t0 = cl["h"], cl["k"], cl["t0"]
        seg_src = pre["srcpad"][core, h][t0 * 128:(t0 + k) * 128]
        idx_img[:, ci * CPC: ci * CPC + (k * 128) // 16] = \
            seg_src.reshape(-1, 16).T.astype(np.int16)
    for h in range(2):
        off = 0 if h == 0 else ht[0]
        cols = pre["colpad"][core, h]
        for t in range(ht[h]):
            dstloc[:, off + t] = cols[t * 128:(t + 1) * 128]
    return np.tile(idx_img, (8, 1)), dstloc


def _build_bass(calls, bt, ht, ncalls_cols, tiles_total):
    import concourse.bass as bass
    import concourse.bacc as bacc
    import concourse.mybir as mybir
    import concourse.tile as tile

    FP32 = mybir.dt.float32
    BF16 = mybir.dt.bfloat16
    FP8 = mybir.dt.float8e4
    I16 = mybir.dt.int16
    AL = mybir.AluOpType
    AF = mybir.ActivationFunctionType

    nc = bacc.Bacc("TRN2", target_bir_lowering=False, debug=False,
                   enable_asserts=False, num_devices=NCORES, num_swdge_queues=4,
                   dynamic_dma_scratch_size=32768)

    x0T = nc.dram_tensor("x0T", [128, SP], BF16, kind="ExternalInput")
    x1T = nc.dram_tensor("x1T", [128, SP], BF16, kind="ExternalInput")
    wl0 = nc.dram_tensor("wl0", [128, 128], BF16, kind="ExternalInput")
    wr0 = nc.dram_tensor("wr0", [128, 128], BF16, kind="ExternalInput")
    wl1 = nc.dram_tensor("wl1", [128, 128], BF16, kind="ExternalInput")
    wr1 = nc.dram_tensor("wr1", [128, 128], BF16, kind="ExternalInput")
    wlm = nc.dram_tensor("wlm", [256, 256], BF16, kind="ExternalInput")
    wrm = nc.dram_tensor("wrm", [256, 256], BF16, kind="ExternalInput")
    wlo = nc.dram_tensor("wlo", [256, 64], BF16, kind="ExternalInput")
    wro = nc.dram_tensor("wro", [256, 64], BF16, kind="ExternalInput")
    b01d = nc.dram_tensor("b01c", [128, 2], FP32, kind="ExternalInput")
    bmd = nc.dram_tensor("bmc", [128, 2], FP32, kind="ExternalInput")
    bod = nc.dram_tensor("bo", [1, 64], FP32, kind="ExternalInput")
    idxd = nc.dram_tensor("idx", [128, ncalls_cols], I16, kind="ExternalInput")
    dstlbd = nc.dram_tensor("dstlb", [128, tiles_total], BF16, kind="ExternalInput")
    invrd = nc.dram_tensor("invrep", [128, SP], BF16, kind="ExternalInput")
    invcd = nc.dram_tensor("invcol", [128, NBLK], FP32, kind="ExternalInput")
    outd = nc.dram_tensor("out", [S, DO], FP32, kind="ExternalOutput")

    with tile.TileContext(nc) as tc:
        with (
            tc.tile_pool(name="const", bufs=1) as cp,
            tc.tile_pool(name="acts", bufs=1) as hp,
            tc.tile_pool(name="g", bufs=8) as gp,
            tc.tile_pool(name="g3", bufs=4) as gp3,
            tc.tile_pool(name="oh", bufs=8) as ohp,
            tc.tile_pool(name="xs", bufs=2) as xsp,
            tc.tile_pool(name="ps1", bufs=1, space="PSUM") as psp1,
            tc.tile_pool(name="ps2", bufs=2, space="PSUM") as psp2,
            tc.tile_pool(name="psy", bufs=2, space="PSUM") as psyp,
            tc.tile_pool(name="ev", bufs=2) as evp,
            tc.tile_pool(name="dram", bufs=1, space="DRAM") as dp,
        ):
            def load(name, dt_, shape, src):
                t = cp.tile(shape, dt_, name=name)
                nc.sync.dma_start(out=t[:], in_=src)
                return t

            wl0t = load("wl0t", BF16, [128, 128], wl0[:])
            wr0t = load("wr0t", BF16, [128, 128], wr0[:])
            wl1t = load("wl1t", BF16, [128, 128], wl1[:])
            wr1t = load("wr1t", BF16, [128, 128], wr1[:])
            wlmt = [load(f"wlmt{i}", BF16, [128, 256], wlm[i * 128:(i + 1) * 128, :]) for i in range(2)]
            wrmt = [load(f"wrmt{i}", BF16, [128, 256], wrm[i * 128:(i + 1) * 128, :]) for i in range(2)]
            wlot = [load(f"wlot{i}", BF16, [128, 64], wlo[i * 128:(i + 1) * 128, :]) for i in range(2)]
            wrot = [load(f"wrot{i}", BF16, [128, 64], wro[i * 128:(i + 1) * 128, :]) for i in range(2)]
            b01t = load("b01t", FP32, [128, 2], b01d[:])
            bmt = load("bmt", FP32, [128, 2], bmd[:])
            bot = load("bot", FP32, [1, 64], bod[:])
            idxt = load("idxt", I16, [128, ncalls_cols], idxd[:])
            dstlb = load("dstlbt", BF16, [128, tiles_total], dstlbd[:])
            invrep = load("invrept", BF16, [128, SP], invrd[:])
            invcol = load("invcolt", FP32, [128, NBLK], invcd[:])

            ones_r = cp.tile([1, 128], FP32, name="ones_r")
            nc.vector.memset(ones_r[:], 1.0)
            iota_i = cp.tile([128, 128], mybir.dt.int32, name="iota_i")
            nc.gpsimd.iota(iota_i[:], pattern=[[1, 128]], base=0,
                           channel_multiplier=0)
            iota_b = cp.tile([128, TPC, 128], BF16, name="iota_b")
            for t in range(TPC):
                nc.vector.tensor_copy(out=iota_b[:, t, :], in_=iota_i[:])

            x0f = hp.tile([128, SP], BF16, name="x0f")
            nc.sync.dma_start(out=x0f[:], in_=x0T[:])
            x1f = hp.tile([128, SP], BF16, name="x1f")
            nc.sync.dma_start(out=x1f[:], in_=x1T[:])
            hT = [hp.tile([128, SP], BF16, name=f"hT{i}") for i in range(2)]
            h2T = [hp.tile([128, SP], BF16, name=f"h2T{i}") for i in range(2)]
            accA = [hp.tile([128, SP], BF16, name=f"accA{i}") for i in range(2)]
            acc3 = hp.tile([128, NBLK * DO], FP32, name="acc3")

            shared = "Shared" if NCORES > 4 else "Local"
            SPG = [SPA, SPB]
            y01_own = [dp.tile([SPG[h], D1], FP8, name=f"y01_own{h}") for h in range(2)]
            Y01 = [dp.tile([HROWS[h], D1], FP8, name=f"Y01{h}", addr_space=shared) for h in range(2)]
            ym_own = [dp.tile([SPG[h], DM], FP8, name=f"ym_own{h}") for h in range(2)]
            Ym = [dp.tile([HROWS[h], DM], FP8, name=f"Ym{h}", addr_space=shared) for h in range(2)]
            DOP = 128   # L3 rows padded to 128 bf16 cols = 256B gather elems
            yo_own = [dp.tile([SPG[h], DOP], BF16, name=f"yo_own{h}") for h in range(2)]
            Yo = [dp.tile([HROWS[h], DOP], BF16, name=f"Yo{h}", addr_space=shared) for h in range(2)]

            def write_y(dsts, b, src_tile, d):
                r0 = b * 128
                if r0 + 128 <= SPA:
                    nc.sync.dma_start(out=dsts[0][r0:r0 + 128, 0:d], in_=src_tile[:])
                elif r0 >= SPA:
                    nc.sync.dma_start(out=dsts[1][r0 - SPA:r0 - SPA + 128, 0:d], in_=src_tile[:])
                else:
                    nlo = SPA - r0
                    nc.sync.dma_start(out=dsts[0][r0:SPA, 0:d], in_=src_tile[0:nlo, :])
                    nc.sync.dma_start(out=dsts[1][0:128 - nlo, 0:d], in_=src_tile[nlo:128, :])

            RG = [list(range(NCORES))]

            def make_ag(src, dst):
                # bitcast to a wider dtype: the collective is element-sliced,
                # so fewer/wider elements move the same bytes in less time
                def f():
                    nc.gpsimd.collective_compute(
                        "AllGather", AL.bypass, replica_groups=RG,
                        ins=[src[:]], outs=[dst[:]])
                return f

            def blk_sl(b):
                return slice(b * 128, (b + 1) * 128)

            def onehot_call(cl, dt):
                k = cl["k"]
                d0 = (0 if cl["h"] == 0 else ht[0]) + cl["t0"]
                oh = ohp.tile([128, TPC, 128], dt, name="oh",
                              tag="oh8" if dt == FP8 else "ohb",
                              padded_shape=[128, TPC, 128])
                nc.vector.tensor_tensor(
                    out=oh[:, 0:k, :], in0=iota_b[:, 0:k, :],
                    in1=dstlb[:, d0:d0 + k].to_broadcast([128, k, 128]),
                    op=AL.is_equal)
                return oh

            # ================= L1 pre: y01_own = [x0@Wl0 | x1@Wl1] =========
            for b in range(NBLK):
                py = psyp.tile([128, 256], FP32, name="py", tag="py")
                nc.tensor.matmul(py[:, 0:128], lhsT=x0f[:, blk_sl(b)], rhs=wl0t[:], start=True, stop=True)
                nc.tensor.matmul(py[:, 128:256], lhsT=x1f[:, blk_sl(b)], rhs=wl1t[:], start=True, stop=True)
                evy = evp.tile([128, 256], FP8, name="evy", tag="evy")
                nc.vector.tensor_copy(out=evy[:], in_=py[:])
                write_y(y01_own, b, evy, D1)
                if b == AGBLK:
                    make_ag(y01_own[0], Y01[0])()
            make_ag(y01_own[1], Y01[1])()

            # ================= two-pass aggregation layer (L1/L2) ============
            def agg_layer(Ytab, wr_tiles, bias_t, h_src, h_dst, wl_next, y_next,
                          d_next, ynext_dt, ag_lo, ag_hi):
                gtiles = {}
                ohs = {}
                emitted = set()

                def emit_gathers(tl):
                    for ci in sorted({c for c, _, _ in tl}):
                        if ci in emitted:
                            continue
                        emitted.add(ci)
                        cl = calls[ci]
                        k = cl["k"]
                        g = gp.tile([128, TPC, D1], FP8, name="g", tag="g")
                        nc.gpsimd.dma_gather(
                            out_ap=g[:, 0:k, :],
                            in_ap=Ytab[cl["h"]][:],
                            idxs_ap=idxt[:, ci * CPC: ci * CPC + (k * 128) // 16],
                            num_idxs=k * 128, num_idxs_reg=k * 128,
                            elem_size=D1, queue_num=0)
                        gtiles[ci] = g
                        ohs[ci] = onehot_call(cl, FP8)

                # ---- pass A: group-a unnormalized sums -> accA (bf16) ----
                for b in range(NBLK):
                    tl = bt[0][b]
                    emit_gathers(tl)
                    psA0 = psp1.tile([128, 128], FP32, name="psA0", tag="a0",
                                     padded_shape=[128, 512])
                    psA1 = psp1.tile([128, 128], FP32, name="psA1", tag="a1",
                                     padded_shape=[128, 512])
                    for n, (ci, slot, dcol) in enumerate(tl):
                        g = gtiles[ci]
                        oh = ohs[ci]
                        first, last = (n == 0), (n == len(tl) - 1)
                        nc.tensor.matmul(psA0[:], lhsT=g[:, slot, 0:128], rhs=oh[:, slot, :],
                                         start=first, stop=last)
                        nc.tensor.matmul(psA1[:], lhsT=g[:, slot, 128:256], rhs=oh[:, slot, :],
                                         start=first, stop=last)
                    nc.vector.tensor_copy(out=accA[0][:, blk_sl(b)], in_=psA0[:])
                    nc.vector.tensor_copy(out=accA[1][:, blk_sl(b)], in_=psA1[:])

                # ---- pass B: group-b sums + merge + y_next ----
                for b in range(NBLK):
                    tl = bt[1][b]
                    emit_gathers(tl)
                    psB0 = psp2.tile([128, 128], FP32, name="psB0", tag="b0",
                                     padded_shape=[128, 512])
                    psB1 = psp2.tile([128, 128], FP32, name="psB1", tag="b1",
                                     padded_shape=[128, 512])
                    psWa = psp1.tile([128, 128], FP32, name="psWa", tag="a0",
                                     padded_shape=[128, 512])
                    psWb = psp1.tile([128, 128], FP32, name="psWb", tag="a1",
                                     padded_shape=[128, 512])
                    ps0, ps1, psW0, psW1 = psB0[:], psB1[:], psWa[:], psWb[:]
                    for n, (ci, slot, dcol) in enumerate(tl):
                        g = gtiles[ci]
                        oh = ohs[ci]
                        first, last = (n == 0), (n == len(tl) - 1)
                        nc.tensor.matmul(ps0, lhsT=g[:, slot, 0:128], rhs=oh[:, slot, :],
                                         start=first, stop=last)
                        nc.tensor.matmul(ps1, lhsT=g[:, slot, 128:256], rhs=oh[:, slot, :],
                                         start=first, stop=last)
                    if h_src is None:
                        nc.tensor.matmul(psW0, lhsT=wr0t[:], rhs=x0f[:, blk_sl(b)], start=True, stop=True)
                        nc.tensor.matmul(psW1, lhsT=wr1t[:], rhs=x1f[:, blk_sl(b)], start=True, stop=True)
                    else:
                        hs = [h_src[0][:, blk_sl(b)], h_src[1][:, blk_sl(b)]]
                        nc.tensor.matmul(psW0, lhsT=wr_tiles[0][:, 0:128], rhs=hs[0], start=True, stop=False)
                        nc.tensor.matmul(psW0, lhsT=wr_tiles[1][:, 0:128], rhs=hs[1], start=False, stop=True)
                        nc.tensor.matmul(psW1, lhsT=wr_tiles[0][:, 128:256], rhs=hs[0], start=True, stop=False)
                        nc.tensor.matmul(psW1, lhsT=wr_tiles[1][:, 128:256], rhs=hs[1], start=False, stop=True)
                    # merge: h = relu((sumA+sumB)*invd + Wr-part + bias)
                    # (DVE may read only ONE psum input per op -> SBUF temp)
                    for ps, psW, acc, half_i in ((ps0, psW0, accA[0], 0), (ps1, psW1, accA[1], 1)):
                        nc.vector.tensor_tensor(out=ps, in0=ps, in1=acc[:, blk_sl(b)], op=AL.add)
                        tmpM = evp.tile([128, 128], FP32, name="tmpM", tag="mg")
                        nc.vector.tensor_tensor(out=tmpM[:], in0=ps, in1=invrep[:, blk_sl(b)], op=AL.mult)
                        nc.vector.tensor_tensor(out=tmpM[:], in0=psW, in1=tmpM[:], op=AL.add)
                        nc.scalar.activation(h_dst[half_i][:, blk_sl(b)], tmpM[:], AF.Relu,
                                             bias=bias_t[:, half_i:half_i + 1])
                    pyn = psyp.tile([128, d_next], FP32, name="pyn", tag="py",
                                    padded_shape=[128, 256])
                    nc.tensor.matmul(pyn[:], lhsT=h_dst[0][:, blk_sl(b)], rhs=wl_next[0][:],
                                     start=True, stop=False)
                    nc.tensor.matmul(pyn[:], lhsT=h_dst[1][:, blk_sl(b)], rhs=wl_next[1][:],
                                     start=False, stop=True)
                    evn = evp.tile([128, d_next], ynext_dt, name="evn",
                                   tag="evy" if ynext_dt == FP8 else "evyf",
                                   padded_shape=[128, 256])
                    nc.vector.tensor_copy(out=evn[:], in_=pyn[:])
                    write_y(y_next, b, evn, d_next)
                    if b == AGBLK:
                        ag_lo()
                ag_hi()

            agg_layer(Y01, None, b01t, None, hT, wlmt, ym_own, DM, FP8,
                      ag_lo=make_ag(ym_own[0], Ym[0]), ag_hi=make_ag(ym_own[1], Ym[1]))
            agg_layer(Ym, wrmt, bmt, hT, h2T, wlot, yo_own, DO, BF16,
                      ag_lo=make_ag(yo_own[0], Yo[0]), ag_hi=make_ag(yo_own[1], Yo[1]))

            # ================= L3: out[node, 64], two-pass ==================
            gtiles3 = {}
            ohs3 = {}
            emitted3 = set()

            def emit_gathers3(tl):
                for ci in sorted({c for c, _, _ in tl}):
                    if ci in emitted3:
                        continue
                    emitted3.add(ci)
                    cl = calls[ci]
                    k = cl["k"]
                    g3 = gp3.tile([128, TPC, DOP], BF16, name="g3", tag="g3")
                    nc.gpsimd.dma_gather(
                        out_ap=g3[:, 0:k, :], in_ap=Yo[cl["h"]][:],
                        idxs_ap=idxt[:, ci * CPC: ci * CPC + (k * 128) // 16],
                        num_idxs=k * 128, num_idxs_reg=k * 128,
                        elem_size=DOP, queue_num=0)
                    gtiles3[ci] = g3
                    ohs3[ci] = onehot_call(cl, BF16)

            # pass A: group-a sums -> acc3 (fp32)
            for b in range(NBLK):
                tl = bt[0][b]
                emit_gathers3(tl)
                psA3t = psp1.tile([128, 128], FP32, name="psA3", tag="a0",
                                  padded_shape=[128, 512])
                psA3 = psA3t[:, 0:DO]
                for n, (ci, slot, dcol) in enumerate(tl):
                    nc.tensor.matmul(psA3, lhsT=ohs3[ci][:, slot, :],
                                     rhs=gtiles3[ci][:, slot, 0:DO],
                                     start=(n == 0), stop=(n == len(tl) - 1))
                nc.vector.tensor_copy(out=acc3[:, b * DO:(b + 1) * DO], in_=psA3)

            # pass B: group-b sums + merge + output
            for b in range(NBLK):
                tl = bt[1][b]
                emit_gathers3(tl)
                ps3t = psp2.tile([128, 128], FP32, name="ps3t", tag="b0",
                                 padded_shape=[128, 512])
                psW3t = psp2.tile([128, 128], FP32, name="psW3t", tag="b1",
                                  padded_shape=[128, 512])
                ps3, psW3 = ps3t[:, 0:DO], psW3t[:, 0:DO]
                for n, (ci, slot, dcol) in enumerate(tl):
                    nc.tensor.matmul(ps3, lhsT=ohs3[ci][:, slot, :],
                                     rhs=gtiles3[ci][:, slot, 0:DO],
                                     start=(n == 0), stop=(n == len(tl) - 1))
                nc.tensor.matmul(psW3, lhsT=h2T[0][:, blk_sl(b)], rhs=wrot[0][:],
                                 start=True, stop=False)
                nc.tensor.matmul(psW3, lhsT=h2T[1][:, blk_sl(b)], rhs=wrot[1][:],
                                 start=False, stop=False)
                nc.tensor.matmul(psW3, lhsT=ones_r[0:1, :], rhs=bot[0:1, :],
                                 start=False, stop=True)
                nc.vector.tensor_tensor(out=ps3, in0=ps3,
                                        in1=acc3[:, b * DO:(b + 1) * DO], op=AL.add)
                # mean normalize: invdeg is per-partition here (dst on partitions)
                tmp3 = evp.tile([128, DO], FP32, name="tmp3", tag="osb")
                nc.scalar.activation(tmp3[:], ps3, AF.Copy,
                                     scale=invcol[:, b:b + 1])
                osb = evp.tile([128, DO], FP32, name="osb", tag="osb2")
                nc.vector.tensor_tensor(out=osb[:], in0=tmp3[:], in1=psW3, op=AL.add)
                rows = min(128, S - b * 128)
                nc.sync.dma_start(out=outd[b * 128: b * 128 + rows, :],
                                  in_=osb[0:rows, :])

    # Post-scheduling pass: set each gather's SWDGE queue to its Tile-assigned
    # DMASW sem lane (mod num queues) -- guarantees the sem<->queue consistency
    # the ucode requires while spreading calls across all 4 descriptor rings.
    import re as _re
    for blk in nc.m.functions[0].blocks:
        for ins in blk.instructions:
            if isinstance(ins, mybir.InstDMAGatherAnt):
                nm = ins.sync_info.on_update[0].ant_name
                ins.queue_num = int(_re.match(r"DMASW(\d+)", nm).group(1)) % 4

    nc.finalize()
    return nc


_CACHE = {}


def _make_inmaps(inputs, pre, calls, ht):
    import ml_dtypes as _ml
    BF = _ml.bfloat16
    x0 = np.asarray(inputs["x0"], np.float32)
    x1 = np.asarray(inputs["x1"], np.float32)
    deg = pre["deg"]
    in_maps = []
    for c in range(NCORES):
        invd_local = (1.0 / np.maximum(deg[c], 1.0)).astype(np.float32)
        invd_pad = np.zeros(SP, np.float32)
        invd_pad[:S] = invd_local
        idx_img, dstloc = _idx_arrays(pre, calls, ht, c)
        x0c = np.zeros((128, SP), BF)
        x0c[:, :S] = x0[c * S:(c + 1) * S, :].T.astype(BF)
        x1c = np.zeros((128, SP), BF)
        x1c[:, :S] = x1[c * S:(c + 1) * S, :].T.astype(BF)
        b01 = np.stack([np.asarray(inputs["b0"], np.float32),
                        np.asarray(inputs["b1"], np.float32)], axis=1)
        bmc = np.asarray(inputs["bm"], np.float32).reshape(2, 128).T.copy()
        in_maps.append({
            "x0T": x0c, "x1T": x1c,
            "wl0": np.asarray(inputs["Wl0"], np.float32).astype(BF),
            "wr0": np.asarray(inputs["Wr0"], np.float32).astype(BF),
            "wl1": np.asarray(inputs["Wl1"], np.float32).astype(BF),
            "wr1": np.asarray(inputs["Wr1"], np.float32).astype(BF),
            "wlm": np.asarray(inputs["Wlm"], np.float32).astype(BF),
            "wrm": np.asarray(inputs["Wrm"], np.float32).astype(BF),
            "wlo": np.asarray(inputs["Wlo"], np.float32).astype(BF),
            "wro": np.asarray(inputs["Wro"], np.float32).astype(BF),
            "b01c": b01,
            "bmc": bmc,
            "bo": np.asarray(inputs["bo"], np.float32)[None, :],
            "idx": idx_img,
            "dstlb": dstloc.astype(BF),
            "invrep": np.tile(invd_pad[None, :], (128, 1)).astype(BF),
            "invcol": invd_pad.reshape(NBLK, 128).T.copy(),
        })
    return in_maps


def _get_program(edge_index):
    if "prog" in _CACHE:
        return _CACHE["prog"]
    pre = _preprocess(edge_index)
    calls, bt, ht = _build_callplan(pre["tiles_hb"])
    tiles_total = ht[0] + ht[1]
    nc = _build_bass(calls, bt, ht, len(calls) * CPC, tiles_total)
    _CACHE["prog"] = (nc, pre, calls, ht)
    return _CACHE["prog"]


LAST_EXEC_NS = None


def kernel(**inputs):
    global LAST_EXEC_NS
    _install_hooks()
    from concourse.bass_utils import run_bass_kernel_spmd

    nc, pre, calls, ht = _get_program(inputs["edge_index"])
    in_maps = _make_inmaps(inputs, pre, calls, ht)
    trace = os.environ.get("KERNEL_TRACE", "0") == "1"
    res = run_bass_kernel_spmd(nc, in_maps, list(range(NCORES)), trace=trace)
    LAST_EXEC_NS = res.exec_time_ns
    return np.concatenate([np.asarray(res.results[c]["out"]) for c in range(NCORES)], axis=0)
